# revision 1
# baseline (speedup 1.0000x reference)
"""Dense transformer block (B=4,S=2048,E=1024,H=16) on 8 trn2 cores.

Sharding: 2 cores per batch sequence; core parity p takes rows p, p+2, ...
(stride-2 interleave) as its query rows -- this balances causal-attention
work exactly across cores.  Each core's x input is row-permuted to
[q rows (local order), other-parity rows] so every SBUF/DRAM offset in the
SPMD program is compile-time constant; causality is enforced with per-core
0/1 mask tensors (pure data).

Datapath: weights and activations in bf16 (accumulation in fp32 PSUM;
residuals and layernorm statistics in fp32).  K^T, V, and the attention
output all stay resident in SBUF -- no DRAM round trips.  FFN streams each
weight byte exactly once.  Softmax EXP is software-pipelined against the
score/AV matmuls, and the per-headgroup normalization epilogue is deferred
into the next head-group so the PE never waits on it.
"""

import numpy as np

B, S, E, H, DH = 4, 2048, 1024, 16, 64
EPS = 1e-5
QR = S // 2          # q rows per core
CH = 512             # q-chunk (matmul free dim)
NCH = QR // CH       # 2 chunks
NKB = S // 128       # 16 key blocks
ET = E // 128        # 8 E tiles
NPR = H // 2         # 8 head pairs
FE = 4 * E           # ffn hidden
NS4 = FE // 128      # 32 ffn hidden slices
SC = 1.0 / np.sqrt(DH)

# feature flags (bisection)
APPROX_OK = False   # use reciprocal_approx_fast (custom DVE) in FAST_RECIP path
DEFER = True      # software-pipeline AV + defer epilogue into next group
FAST_RECIP = True # reciprocal_approx_fast + packed sel broadcast

_PROG = None


def _visits(ch):
    """per q-chunk: list of (key_block, qlo or None) in accumulation order"""
    if ch == 0:
        return [(kb, 128 * (kb % 8)) for kb in (0, 1, 2, 3, 8, 9, 10, 11)]
    full = [(kb, None) for kb in (0, 1, 2, 3, 8, 9, 10, 11)]
    diag = [(kb, 128 * ((kb - 4) if kb < 8 else (kb - 12)))
            for kb in (4, 5, 6, 7, 12, 13, 14, 15)]
    return full + diag


def _build():
    import concourse.bacc as bacc
    import concourse.tile as tile
    from concourse import mybir
    from concourse.masks import make_identity

    F32 = mybir.dt.float32
    F32R = mybir.dt.float32r
    BF16 = mybir.dt.bfloat16
    AF = mybir.ActivationFunctionType

    def r(ap):  # float32r view for fp32 matmul operands
        return ap.bitcast(F32R)

    nc = bacc.Bacc("TRN2", target_bir_lowering=False, debug=False, num_devices=8)

    xin = nc.dram_tensor("xin", [S, E], F32, kind="ExternalInput").ap()
    masks = nc.dram_tensor("masks", [2, 128, 256], BF16, kind="ExternalInput").ap()
    sel = nc.dram_tensor("sel", [128, 128], BF16, kind="ExternalInput").ap()
    wq = nc.dram_tensor("wq", [H, E, DH], BF16, kind="ExternalInput").ap()
    wk = nc.dram_tensor("wk", [H, E, DH], BF16, kind="ExternalInput").ap()
    wv = nc.dram_tensor("wv", [H, E, DH], BF16, kind="ExternalInput").ap()
    wo = nc.dram_tensor("wo", [E, E], BF16, kind="ExternalInput").ap()
    bo = nc.dram_tensor("bo", [E], BF16, kind="ExternalInput").ap()
    ln1g = nc.dram_tensor("ln1g", [E], F32, kind="ExternalInput").ap()
    ln1b = nc.dram_tensor("ln1b", [E], F32, kind="ExternalInput").ap()
    ln2g = nc.dram_tensor("ln2g", [E], F32, kind="ExternalInput").ap()
    ln2b = nc.dram_tensor("ln2b", [E], F32, kind="ExternalInput").ap()
    w1 = nc.dram_tensor("w1", [E, FE], BF16, kind="ExternalInput").ap()
    b1 = nc.dram_tensor("b1", [FE], F32, kind="ExternalInput").ap()
    w2 = nc.dram_tensor("w2", [FE, E], BF16, kind="ExternalInput").ap()
    b2 = nc.dram_tensor("b2", [E], BF16, kind="ExternalInput").ap()
    out = nc.dram_tensor("out", [QR, E], F32, kind="ExternalOutput").ap()

    with tile.TileContext(nc, pool_alloc_mode="queue") as tc:
        consts = tc.alloc_tile_pool(name="consts", bufs=1)
        work = tc.alloc_tile_pool(name="work", bufs=3)
        small = tc.alloc_tile_pool(name="small", bufs=6)

        ident = consts.tile([128, 128], F32)
        make_identity(nc, ident)
        identr = consts.tile([128, 128], F32R, tag="identr")
        nc.vector.tensor_copy(identr, ident)
        onesb = consts.tile([128, 256], BF16, tag="onesb")
        nc.vector.memset(onesb, 1.0)
        onesf = consts.tile([128, 128], F32, tag="onesf")
        nc.vector.memset(onesf, 1.0)
        onesr = consts.tile([128, 128], F32R, tag="onesr")
        nc.vector.tensor_copy(onesr, onesf)
        epst = consts.tile([128, 1], F32)
        nc.vector.memset(epst, EPS)
        sel_sb = consts.tile([128, 128], BF16, tag="sel")
        nc.sync.dma_start(sel_sb, sel)
        ln1g_sb = consts.tile([128, ET], F32, tag="lnp1")
        nc.sync.dma_start(ln1g_sb, ln1g.rearrange("(eo ei) -> ei eo", ei=128))
        ln1b_sb = consts.tile([128, ET], F32, tag="lnp2")
        nc.sync.dma_start(ln1b_sb, ln1b.rearrange("(eo ei) -> ei eo", ei=128))
        ln2g_sb = consts.tile([128, ET], F32, tag="lnp3")
        nc.sync.dma_start(ln2g_sb, ln2g.rearrange("(eo ei) -> ei eo", ei=128))
        ln2b_sb = consts.tile([128, ET], F32, tag="lnp4")
        nc.sync.dma_start(ln2b_sb, ln2b.rearrange("(eo ei) -> ei eo", ei=128))
        bo_sb = consts.tile([1, E], BF16, tag="bo")
        nc.sync.dma_start(bo_sb, bo[None, :])
        b2_sb = consts.tile([1, E], BF16, tag="b2")
        nc.sync.dma_start(b2_sb, b2[None, :])
        b1_sb = consts.tile([128, NS4], F32, tag="b1")
        nc.sync.dma_start(b1_sb, b1.rearrange("(so si) -> si so", si=128))
        wedges = []
        for w in range(2):
            mt = consts.tile([128, 256], BF16, tag=f"mask{w}", name=f"wedge{w}")
            nc.sync.dma_start(mt, masks[w])
            wedges.append(mt)
        rsums = consts.tile([128, 512], F32, tag="rsums")
        nc.vector.memset(rsums, 1.0)  # rows off {0,32,64,96} stay 1.0 (benign)

        def layernorm_rows(x_tiles, n_tiles, nrow_tiles):
            """natural-layout LN stats+center+scale for a list of row tiles"""
            for j in range(nrow_tiles):
                xt = x_tiles[j]
                st = small.tile([128, 2, 6], F32, tag="bnst")
                xr = xt.rearrange("p (a b) -> p a b", a=2)
                for sg in range(2):
                    nc.vector.bn_stats(st[:, sg, :], xr[:, sg, :])
                mv = small.tile([128, 2], F32, tag="bnmv")
                nc.vector.bn_aggr(mv, st)
                rstd = small.tile([128, 1], F32, tag="rstd")
                nc.scalar.activation(rstd, mv[:, 1:2], AF.Sqrt, bias=epst)
                nc.vector.reciprocal(rstd, rstd)
                nc.vector.tensor_scalar(
                    n_tiles[j], xt, mv[:, 0:1], rstd,
                    mybir.AluOpType.subtract, mybir.AluOpType.mult,
                )

        # ---------------- persistent SBUF tensors ----------------
        x2_pool = tc.alloc_tile_pool(name="x2", bufs=1)
        X2 = x2_pool.tile([128, ET, E], F32, tag="x2", name="X2")  # x + attn out
        att_pool = tc.alloc_tile_pool(name="att", bufs=NPR)
        QT = [att_pool.tile([128, QR], BF16, tag="qt", name=f"QT{i}") for i in range(NPR)]
        KT = [att_pool.tile([128, S], BF16, tag="kt", name=f"KT{i}") for i in range(NPR)]
        VSB = att_pool.tile([128, NKB, H, DH + 1], BF16, tag="vsb", name="VSB",
                            bufs=1)
        oac_pool = tc.alloc_tile_pool(name="oac", bufs=NPR)
        OACC = [oac_pool.tile([128, QR], BF16, tag="oacc", name=f"OACC{i}") for i in range(NPR)]
        # ones column for the rowsum rider
        nc.vector.tensor_copy(VSB[:, :, :, DH],
                              onesb.rearrange("p (a b) -> p a b", a=NKB))

        # ---------------- P0/P1: LN1 -> y1T; QKV projections ----------------
        with (
            tc.tile_pool(name="y1t", bufs=ET) as y1t_pool,
            tc.tile_pool(name="wstr", bufs=2) as wstr,
            tc.tile_pool(name="psP1", bufs=2, space="PSUM") as psP1,
            tc.tile_pool(name="psT", bufs=2, space="PSUM") as psT,
        ):
            Y1T = [y1t_pool.tile([128, S], BF16, tag="y1t", name=f"Y1T{i}") for i in range(ET)]
            # LN1 + transpose, in groups of 2 row-tiles
            with tc.tile_pool(name="xtn", bufs=2) as xtn:
                for tq in range(S // 256):
                    xts, n1s = [], []
                    for j in range(2):
                        ri = tq * 2 + j
                        xt = xtn.tile([128, E], F32, tag="xt", name=f"xt{j}", bufs=2)
                        nc.sync.dma_start(xt, xin[ri * 128:(ri + 1) * 128, :])
                        xts.append(xt)
                        n1s.append(xtn.tile([128, E], F32R, tag="n1", name=f"n1s{j}"))
                    layernorm_rows(xts, n1s, 2)
                    for e in range(ET):
                        ps = psT.tile([128, 256], F32R, tag="pst")
                        for j in range(2):
                            nc.tensor.transpose(
                                ps[:, j * 128:(j + 1) * 128],
                                n1s[j][:, e * 128:(e + 1) * 128], identr)
                        nc.scalar.activation(
                            Y1T[e][:, tq * 256:(tq + 1) * 256],
                            ps.bitcast(F32), AF.Identity,
                            bias=ln1b_sb[:, e:e + 1],
                            scale=ln1g_sb[:, e:e + 1])

            # K projection -> KT (sbuf), Q projection -> QT (sbuf)
            for pr in range(NPR):
                wk_sb = wstr.tile([128, ET, 2, DH], BF16, tag="wqk")
                for h2 in range(2):
                    nc.sync.dma_start(
                        wk_sb[:, :, h2, :], wk[2 * pr + h2].rearrange(
                            "(eo ei) d -> ei eo d", ei=128))
                for kc in range(S // 512):
                    ps = psP1.tile([128, 512], F32, tag="proj")
                    for e in range(ET):
                        nc.tensor.matmul(
                            ps, wk_sb[:, e, :],
                            Y1T[e][:, kc * 512:(kc + 1) * 512],
                            start=(e == 0), stop=(e == ET - 1))
                    nc.vector.tensor_copy(KT[pr][:, kc * 512:(kc + 1) * 512], ps)
                wq_sb = wstr.tile([128, ET, 2, DH], BF16, tag="wqk")
                for h2 in range(2):
                    nc.sync.dma_start(
                        wq_sb[:, :, h2, :], wq[2 * pr + h2].rearrange(
                            "(eo ei) d -> ei eo d", ei=128))
                for qc in range(NCH):
                    ps = psP1.tile([128, 512], F32, tag="proj")
                    for e in range(ET):
                        nc.tensor.matmul(
                            ps, wq_sb[:, e, :],
                            Y1T[e][:, qc * 512:(qc + 1) * 512],
                            start=(e == 0), stop=(e == ET - 1))
                    nc.vector.tensor_copy(QT[pr][:, qc * 512:(qc + 1) * 512], ps)

            # V projection (natural layout) -> VSB
            for half in range(2):
                wv_sb = wstr.tile([128, ET, 8, DH], BF16, tag="wv", bufs=1)
                for h8 in range(8):
                    nc.sync.dma_start(
                        wv_sb[:, :, h8, :], wv[8 * half + h8].rearrange(
                            "(eo ei) d -> ei eo d", ei=128))
                for kb in range(NKB):
                    ps = psP1.tile([128, 512], F32, tag="proj")
                    for e in range(ET):
                        nc.tensor.matmul(
                            ps, Y1T[e][:, kb * 128:(kb + 1) * 128],
                            wv_sb[:, e, :],
                            start=(e == 0), stop=(e == ET - 1))
                    nc.vector.tensor_copy(
                        VSB[:, kb, 8 * half:8 * half + 8, 0:DH],
                        ps.rearrange("p (h d) -> p h d", h=8))

        # ---------------- P2: attention ----------------
        with (
            tc.tile_pool(name="nrm", bufs=2) as nrmp,
            tc.tile_pool(name="pt", bufs=4) as ptp,
            tc.tile_pool(name="psS", bufs=2, space="PSUM") as psS,
            tc.tile_pool(name="psO", bufs=4, space="PSUM") as psO,
        ):
            def issue_avs(av):
                ops, hg, kb, q0, pts, first, last = av
                N = CH - q0
                for pi in range(2):
                    for hh in range(2):
                        h = 2 * pi + hh
                        nc.tensor.matmul(
                            ops[h][0:DH + 1, q0:CH],
                            VSB[:, kb, 4 * hg + h, :],
                            pts[pi][:, hh * 512:hh * 512 + N],
                            start=first, stop=last, skip_group_check=True)

            def issue_epilogue(ep):
                ops, prs, ch = ep
                if FAST_RECIP:
                    # extraction only: free the ops banks fast.  rowsum rows
                    # via ACT, unnormalized o via DVE (parallel engines).
                    rows = (0, 32, 64, 96)
                    for h in range(4):
                        eng = nc.scalar.copy if h % 2 else nc.vector.tensor_copy
                        eng(rsums[rows[h]:rows[h] + 1, :],
                            ops[h][DH:DH + 1, :])
                    for h in range(4):
                        pr, odd = prs[h // 2], h % 2
                        nc.vector.tensor_copy(
                            OACC[pr][odd * 64:(odd + 1) * 64,
                                     ch * CH:(ch + 1) * CH],
                            ops[h][0:DH, :])
                    rcp = nrmp.tile([128, 512], F32, tag="rcp")
                    nc.vector.reciprocal(rcp, rsums)
                    return (rcp, prs, ch)
                else:
                    # baseline-shaped: full reciprocal + per-head ones bcast
                    rs4 = nrmp.tile([128, 1024], F32, tag="rs4")
                    rcpos = [(0, 0), (32, 0), (64, 0), (0, 512)]
                    for h in range(4):
                        rw, cl = rcpos[h]
                        nc.vector.tensor_copy(rs4[rw:rw + 1, cl:cl + 512],
                                              ops[h][DH:DH + 1, :])
                    rs4r = nrmp.tile([128, 1024], F32R, tag="rs4r")
                    with nc.allow_low_precision(reason="f32r round for bc"):
                        nc.vector.reciprocal(rs4r, rs4)
                    for h in range(4):
                        pr, odd = prs[h // 2], h % 2
                        rw, cl = rcpos[h]
                        bc = psS.tile([128, 1024], F32, tag="sc")
                        nc.tensor.matmul(
                            bc[0:64, 0:512], onesr[rw:rw + 1, 0:64],
                            rs4r[rw:rw + 1, cl:cl + 512], start=True, stop=True)
                        bcs = nrmp.tile([64, 512], F32, tag="bcs")
                        nc.vector.tensor_copy(bcs, bc[0:64, 0:512])
                        nc.vector.tensor_mul(
                            OACC[pr][odd * 64:(odd + 1) * 64,
                                     ch * CH:(ch + 1) * CH],
                            ops[h][0:DH, :], bcs)
                    return None

            def issue_norm(nm):
                # deferred broadcast (PE, inputs long ready) + in-place scale
                rcp, prs, ch = nm
                rcpb = nrmp.tile([128, 512], BF16, tag="rcpb")
                nc.vector.tensor_copy(rcpb, rcp)
                for half in range(2):
                    pr = prs[half]
                    bc = psS.tile([128, 1024], F32, tag="sc")
                    nc.tensor.matmul(
                        bc[:, 0:512], sel_sb[64 * half:64 * half + 33, :],
                        rcpb[64 * half:64 * half + 33, :],
                        start=True, stop=True)
                    bcs = nrmp.tile([128, 512], BF16, tag="bcs")
                    nc.vector.tensor_copy(bcs, bc[:, 0:512])
                    nc.vector.tensor_mul(
                        OACC[pr][:, ch * CH:(ch + 1) * CH],
                        OACC[pr][:, ch * CH:(ch + 1) * CH], bcs)

            pend_ep = None
            norm_q = []
            for ch in range(NCH):
                visits = _visits(ch)
                for hg in range(4):
                    prs = (2 * hg, 2 * hg + 1)
                    ops = [psO.tile([128, 512], F32, tag="ot", name=f"ot{h}")
                           for h in range(4)]
                    pend_av = None
                    for vi, (kb, qlo) in enumerate(visits):
                        q0 = 0 if qlo is None else qlo
                        N = CH - q0
                        kcol = kb * 128
                        wm = wedges[0 if kb < 8 else 1]
                        pts = []
                        for pi, pr in enumerate(prs):
                            pss = psS.tile([128, 1024], F32, tag="sc")
                            for hh in range(2):
                                nc.tensor.matmul(
                                    pss[:, hh * 512:hh * 512 + N],
                                    KT[pr][hh * 64:(hh + 1) * 64, kcol:kcol + 128],
                                    QT[pr][hh * 64:(hh + 1) * 64,
                                           ch * CH + q0:(ch + 1) * CH],
                                    start=True, stop=True)
                            pt = ptp.tile([128, 1024], BF16, tag="pt")
                            if N == 512:
                                nc.scalar.activation(pt, pss, AF.Exp, scale=SC)
                            else:
                                # strided AP: exp only the 2xN useful columns
                                pt3 = pt.rearrange("p (h c) -> p h c", h=2)
                                ps3 = pss.rearrange("p (h c) -> p h c", h=2)
                                nc.scalar.activation(pt3[:, :, 0:N],
                                                     ps3[:, :, 0:N],
                                                     AF.Exp, scale=SC)
                            if qlo is not None:
                                pt3 = pt.rearrange("p (h c) -> p h c", h=2)
                                nc.vector.tensor_mul(
                                    pt3[:, :, 0:128], pt3[:, :, 0:128],
                                    wm.rearrange("p (a b) -> p a b", a=2))
                            pts.append(pt)
                        if DEFER:
                            if vi == 1 and pend_ep is not None:
                                nm = issue_epilogue(pend_ep)
                                if nm is not None:
                                    norm_q.append(nm)
                                pend_ep = None
                            if vi == 5 and norm_q:
                                issue_norm(norm_q.pop(0))
                            if pend_av is not None:
                                issue_avs(pend_av)
                            pend_av = (ops, hg, kb, q0, pts,
                                       vi == 0, vi == len(visits) - 1)
                        else:
                            issue_avs((ops, hg, kb, q0, pts,
                                       vi == 0, vi == len(visits) - 1))
                    if DEFER:
                        issue_avs(pend_av)
                        pend_ep = (ops, prs, ch)
                    else:
                        nm = issue_epilogue((ops, prs, ch))
                        if nm is not None:
                            issue_norm(nm)
            if DEFER:
                nm = issue_epilogue(pend_ep)
                if nm is not None:
                    norm_q.append(nm)
                for nm in norm_q:
                    issue_norm(nm)

        # ---------------- P3: out projection + residual ----------------
        with (
            tc.tile_pool(name="wop", bufs=1) as wop,
            tc.tile_pool(name="xqp", bufs=3) as xqp,
            tc.tile_pool(name="psP3", bufs=4, space="PSUM") as psP3,
        ):
            wo_sb = wop.tile([128, ET, E], BF16, tag="wo")
            nc.sync.dma_start(wo_sb, wo.rearrange("(po pi) o -> pi po o", pi=128))
            for qt in range(ET):
                xq = xqp.tile([128, E], F32, tag="xq")
                nc.sync.dma_start(xq, xin[qt * 128:(qt + 1) * 128, :])
                for eh in range(2):
                    ps = psP3.tile([128, 512], F32, tag="po")
                    for pr in range(NPR):
                        nc.tensor.matmul(
                            ps, OACC[pr][:, qt * 128:(qt + 1) * 128],
                            wo_sb[:, pr, eh * 512:(eh + 1) * 512],
                            start=(pr == 0), stop=False)
                    nc.tensor.matmul(
                        ps, onesb[0:1, 0:128], bo_sb[0:1, eh * 512:(eh + 1) * 512],
                        start=False, stop=True)
                    nc.vector.tensor_add(
                        X2[:, qt, eh * 512:(eh + 1) * 512], ps,
                        xq[:, eh * 512:(eh + 1) * 512])

        oac_pool.release()
        att_pool.release()

        # ---------------- P4: LN2 -> y2T ----------------
        y2t_pool = tc.alloc_tile_pool(name="y2t", bufs=ET)
        Y2T = [y2t_pool.tile([128, QR], BF16, tag="y2t", name=f"Y2T{i}") for i in range(ET)]
        with (
            tc.tile_pool(name="psT2", bufs=2, space="PSUM") as psT2,
            tc.tile_pool(name="xn2", bufs=4) as xn2,
        ):
            for tq in range(QR // 512):
                x2s = [X2[:, tq * 4 + j, :] for j in range(4)]
                n2s = [xn2.tile([128, E], F32R, tag="n2", name=f"n2s{j}") for j in range(4)]
                layernorm_rows(x2s, n2s, 4)
                for e in range(ET):
                    ps = psT2.tile([128, 512], F32R, tag="pst")
                    for j in range(4):
                        nc.tensor.transpose(
                            ps[:, j * 128:(j + 1) * 128],
                            n2s[j][:, e * 128:(e + 1) * 128], identr)
                    if e % 2:
                        nc.scalar.activation(
                            Y2T[e][:, tq * 512:(tq + 1) * 512],
                            ps.bitcast(F32), AF.Identity,
                            bias=ln2b_sb[:, e:e + 1],
                            scale=ln2g_sb[:, e:e + 1])
                    else:
                        nc.vector.tensor_scalar(
                            Y2T[e][:, tq * 512:(tq + 1) * 512],
                            ps.bitcast(F32),
                            ln2g_sb[:, e:e + 1], ln2b_sb[:, e:e + 1],
                            mybir.AluOpType.mult, mybir.AluOpType.add)

        # ---------------- P5: FFN + residual -> out ----------------
        h1_pool = tc.alloc_tile_pool(name="h1", bufs=NS4)
        h1t = [h1_pool.tile([128, QR], BF16, tag="h1", name=f"h1_{i}") for i in range(NS4)]
        with (
            tc.tile_pool(name="w1s", bufs=3) as w1s,
            tc.tile_pool(name="psF1", bufs=2, space="PSUM") as psF1,
        ):
            for s4 in range(NS4):
                w1_sb = w1s.tile([128, ET, 128], BF16, tag="w1")
                nc.sync.dma_start(
                    w1_sb, w1.rearrange("(eo ei) f -> ei eo f", ei=128)
                    [:, :, s4 * 128:(s4 + 1) * 128])
                for qh in range(NCH):
                    ps = psF1.tile([128, 512], F32, tag="f1")
                    for e in range(ET):
                        nc.tensor.matmul(
                            ps, w1_sb[:, e, :],
                            Y2T[e][:, qh * 512:(qh + 1) * 512],
                            start=(e == 0), stop=(e == ET - 1))
                    nc.scalar.activation(h1t[s4][:, qh * 512:(qh + 1) * 512],
                                         ps, AF.Relu, bias=b1_sb[:, s4:s4 + 1])

        with (
            tc.tile_pool(name="w2s", bufs=3) as w2s,
            tc.tile_pool(name="psF2", bufs=8, space="PSUM") as psF2,
        ):
            for eh in range(2):
                pss = [psF2.tile([128, 512], F32, tag="f2", name=f"f2_{i}")
                       for i in range(ET)]
                for s4 in range(NS4):
                    w2_sb = w2s.tile([128, 512], BF16, tag="w2")
                    nc.sync.dma_start(
                        w2_sb, w2[s4 * 128:(s4 + 1) * 128, eh * 512:(eh + 1) * 512])
                    for qt in range(ET):
                        nc.tensor.matmul(
                            pss[qt], h1t[s4][:, qt * 128:(qt + 1) * 128],
                            w2_sb, start=(s4 == 0), stop=False)
                for qt in range(ET):
                    nc.tensor.matmul(
                        pss[qt], onesb[0:1, 0:128],
                        b2_sb[0:1, eh * 512:(eh + 1) * 512],
                        start=False, stop=True)
                    ot = work.tile([128, 512], F32, tag="stg")
                    nc.vector.tensor_add(ot, pss[qt],
                                         X2[:, qt, eh * 512:(eh + 1) * 512])
                    nc.sync.dma_start(
                        out[qt * 128:(qt + 1) * 128,
                            eh * 512:(eh + 1) * 512], ot)

        h1_pool.release()
        y2t_pool.release()
        x2_pool.release()
        small.release()
        work.release()
        consts.release()

    nc.compile()
    return nc


def _prep_inputs(inputs):
    import ml_dtypes
    BF = ml_dtypes.bfloat16
    x = np.ascontiguousarray(inputs["x"], dtype=np.float32)
    selm = np.zeros((128, 128), np.float32)
    selm[0, 0:64] = 1.0
    selm[32, 64:128] = 1.0
    selm[64, 0:64] = 1.0
    selm[96, 64:128] = 1.0
    shared = {
        "sel": selm.astype(BF),
        "wq": np.ascontiguousarray(inputs["Wq"]).astype(BF),
        "wk": np.ascontiguousarray(inputs["Wk"]).astype(BF),
        "wv": np.ascontiguousarray(inputs["Wv"]).astype(BF),
        "wo": np.ascontiguousarray(inputs["Wo"]).astype(BF),
        "bo": np.ascontiguousarray(inputs["bo"]).astype(BF),
        "ln1g": np.ascontiguousarray(inputs["ln1_g"], np.float32),
        "ln1b": np.ascontiguousarray(inputs["ln1_b"], np.float32),
        "ln2g": np.ascontiguousarray(inputs["ln2_g"], np.float32),
        "ln2b": np.ascontiguousarray(inputs["ln2_b"], np.float32),
        "w1": np.ascontiguousarray(inputs["W1"]).astype(BF),
        "b1": np.ascontiguousarray(inputs["b1"], np.float32),
        "w2": np.ascontiguousarray(inputs["W2"]).astype(BF),
        "b2": np.ascontiguousarray(inputs["b2"]).astype(BF),
    }
    in_maps = []
    for c in range(8):
        b, p = c // 2, c % 2
        perm = np.concatenate([np.arange(p, S, 2), np.arange(1 - p, S, 2)])
        kk = np.arange(128)[:, None]
        qq = np.arange(128)[None, :]
        m = np.zeros((2, 128, 128), np.float32)
        m[0] = (qq >= kk).astype(np.float32)          # own-parity blocks
        if p == 0:
            m[1] = (qq > kk).astype(np.float32)       # other-parity, even core
        else:
            m[1] = (qq >= kk).astype(np.float32)      # other-parity, odd core
        m2 = np.concatenate([m, m], axis=2)           # [2,128,256]: wedge doubled
        im = dict(shared)
        im["xin"] = np.ascontiguousarray(x[b][perm])
        im["masks"] = m2.astype(BF)
        in_maps.append(im)
    return in_maps


def _get_prog():
    global _PROG
    if _PROG is None:
        _PROG = _build()
    return _PROG


def run(inputs, trace=False):
    from concourse.bass_utils import run_bass_kernel_spmd

    nc = _get_prog()
    in_maps = _prep_inputs(inputs)
    kw = {}
    if trace:
        import sys, types
        try:
            from antenv.axon_hooks import get_axon_ntff_profile_hook  # noqa
        except ImportError:
            from trn_agent_boot.trn_boot import _ntff_profile_via_ctypes
            hook = _ntff_profile_via_ctypes("/opt/axon/libaxon_pjrt.so")
            mod = types.ModuleType("antenv.axon_hooks")
            mod.get_axon_ntff_profile_hook = lambda: hook
            sys.modules["antenv.axon_hooks"] = mod
        kw["trace"] = True
    res = run_bass_kernel_spmd(nc, in_maps, core_ids=list(range(8)), **kw)
    x = inputs["x"]
    outp = np.empty((B, S, E), np.float32)
    for c in range(8):
        b, p = c // 2, c % 2
        outp[b, p::2, :] = res.results[c]["out"]
    return outp, res


def kernel(**inputs):
    outp, _ = run(inputs)
    return outp

